# revision 14
# baseline (speedup 1.0000x reference)
"""Multi-head attention (B=2, S=2048, D=1024, H=16, causal, unscaled scores)
on 8 Trainium2 NeuronCores.

Sharding: 2 batches x 4 head-groups (4 heads each). Core c handles batch
c//4, heads 4*(c%4) .. 4*(c%4)+3. Each core computes its group's QKV
projections, causal attention, and a partial output projection
(row-slice of wo); the host sums the 4 partials per batch (the
all-reduce) and adds the bias terms.

Precision: Q/K path fp16 (scores accumulate in fp32 PSUM); V path bf16.
Bias terms bv/bo are folded in exactly on the host (C = U/colsum + 1*bv
since softmax rows sum to 1).

Schedule (PE-density-first; the kernel is TensorE-bound):
  - 20 warmup outer-product matmuls at t~0 hold the HAM activity window
    busy during the DMA prologue so real matmuls start at 2.4 GHz.
  - input loads ride the sync HWDGE ring need-ordered; output stores
    ride the gpsimd SWDGE ring.
  - attnV is emitted with the PROBABILITIES as the stationary operand
    and V (65 cols: 64 dims + ones column for the denominator) as the
    moving operand: per (J, head, q-tile) a 65-cycle matmul accumulates
    u[q, 65] over J in PSUM. This halves attnV PE cycles vs the
    [dh, q-moving] orientation (moving cols 65 vs 512) and lands the
    output q-major, so softmax normalization is a per-partition
    tensor_scalar_mul fused into the PSUM evacuation (no selector
    matmuls, no broadcast multiplies).
  - u banks: one PSUM bank per q-tile holding 4 heads x 65 cols; the
    denominator column is gathered per bank, inverted with the fast
    approx reciprocal, and applied during evacuation to ctq [q, ch].
  - ctq [q, 256] is transposed back to ct [ch, q] with two PE identity
    transposes per q-tile (cheap: 128 moving cols each) for the output
    projection, which is unchanged.
  - diag masking: exp the [r:512] window, then affine_select only the
    128-col diagonal block (the rest of the window is fully valid).
  - projection work is emitted in chunks as PE fillers between
    attention steps; outproj is deferred ~2 slices; IS=3's outproj
    tiles are emitted right after each q-tile's evacuation so the tail
    only waits on the last q-tile.
"""

import numpy as np

D = 1024
S = 2048
NH = 16
DH = 64
B = 2
G = 4            # head-groups = cores per batch
HG = NH // G     # 4 heads per group
GD = HG * DH     # 256 columns per group
KT = D // 128    # 8 k-tiles
MS = S // 512    # 4 m-slices
JT = S // 128    # 16 j-tiles
IST = S // 512   # 4 i-slices

_cached = None


def _build(rep=1, dbg=False):
    from concourse import bacc
    import concourse.mybir as mybir
    import concourse.tile as tile

    f32 = mybir.dt.float32
    f16 = mybir.dt.float16
    bf16 = mybir.dt.bfloat16
    Act = mybir.ActivationFunctionType
    Alu = mybir.AluOpType

    nc = bacc.Bacc(None, target_bir_lowering=False)
    xq = nc.dram_tensor("xq", [D, S], f16, kind="ExternalInput")
    xk = nc.dram_tensor("xk", [D, S], f16, kind="ExternalInput")
    xv = nc.dram_tensor("xv", [D, S], bf16, kind="ExternalInput")
    wqg = nc.dram_tensor("wqg", [D, GD], f16, kind="ExternalInput")
    wkg = nc.dram_tensor("wkg", [D, GD], f16, kind="ExternalInput")
    wvg = nc.dram_tensor("wvg", [D, GD], bf16, kind="ExternalInput")
    wog = nc.dram_tensor("wog", [GD, D], bf16, kind="ExternalInput")
    bqg = nc.dram_tensor("bqg", [2, 128, 1], f32, kind="ExternalInput")
    bkg = nc.dram_tensor("bkg", [2, 128, 1], f32, kind="ExternalInput")
    identg = nc.dram_tensor("identg", [128, 128], bf16, kind="ExternalInput")
    outp = nc.dram_tensor("outp", [S, D], bf16, kind="ExternalOutput")
    ctd = (
        nc.dram_tensor("ctd", [128, 2, S], bf16, kind="ExternalOutput")
        if dbg
        else None
    )

    with tile.TileContext(nc) as tc:
        with (
            tc.tile_pool(name="wpool", bufs=1) as wpool,
            tc.tile_pool(name="xqk", bufs=2) as xqk,
            tc.tile_pool(name="xvs", bufs=3) as xvs,
            tc.tile_pool(name="big", bufs=1) as big,
            tc.tile_pool(name="ppool", bufs=34) as ppool,
            tc.tile_pool(name="small", bufs=6) as small,
            tc.tile_pool(name="ctqp", bufs=3) as ctqp,
            tc.tile_pool(name="osb", bufs=6) as osb,
            tc.tile_pool(name="ps", bufs=2, space="PSUM") as ps,
            tc.tile_pool(name="po", bufs=2, space="PSUM") as po,
            tc.tile_pool(name="psU", bufs=2, space="PSUM") as psU,
        ):
            def emit_body():
                # ---- resident weights / constants ----
                wq_t = wpool.tile([128, KT, GD], f16, tag="wq")
                wk_t = wpool.tile([128, KT, GD], f16, tag="wk")
                wv_t = wpool.tile([128, KT, GD], bf16, tag="wv")
                wo_t = wpool.tile([128, 2, D], bf16, tag="wo")
                bq_t = wpool.tile([128, 2, 1], f32, tag="bq")
                bk_t = wpool.tile([128, 2, 1], f32, tag="bk")
                ident = wpool.tile([128, 128], bf16, tag="ident")
                warm_sink = wpool.tile([1, 16], f32, tag="wsink")

                # tiny first so warmup matmuls can start ~t=0 (row 0 of the
                # identity is its own 256B DMA; the rest follows)
                nc.scalar.dma_start(out=ident[0:1, :], in_=identg[0:1, :])
                nc.scalar.dma_start(out=ident[1:128, :], in_=identg[1:128, :])
                nc.scalar.dma_start(out=bq_t, in_=bqg[:].rearrange("t p o -> p t o"))
                nc.scalar.dma_start(out=bk_t, in_=bkg[:].rearrange("t p o -> p t o"))

                # ---- PE warmup: ~3.5us of junk matmuls so the HAM
                # un-throttles before the first projection matmul ----
                wpsum = po.tile([128, 128], f32, tag="po", name="warm")
                for i in range(20):
                    nc.tensor.matmul(
                        wpsum,
                        ident[0:1, 0:128],
                        ident[0:1, 0:128],
                        start=(i == 0),
                        stop=(i == 19),
                    )
                nc.vector.tensor_copy(warm_sink, wpsum[0:1, 0:16])

                # ---- input streams ----
                # sync ring: wq, xq(m0), wk, xk(m0), then xq/xk m1..3
                # scalar ring: ident/bq/bk, wv, xv(m0), wo, xv m1..3
                nc.sync.dma_start(out=wq_t[:, 0:4, :], in_=xq_like_w(wqg)[:, 0:4, :])

                xq_ts, xk_ts, xv_ts = [], [], []
                xq_r = xq[:].rearrange("(kt p) s -> p kt s", p=128)
                xk_r = xk[:].rearrange("(kt p) s -> p kt s", p=128)
                xv_r = xv[:].rearrange("(kt p) s -> p kt s", p=128)

                def load_m(m):
                    ms = slice(m * 512, (m + 1) * 512)
                    xqt = xqk.tile([128, KT, 512], f16, tag="xq", name="xqt")
                    xkt = xqk.tile([128, KT, 512], f16, tag="xk", name="xkt")
                    xvt = xvs.tile([128, KT, 512], bf16, tag="xv", name="xvt")
                    if m == 0:
                        # need-ordered halves so kk 0-3 matmuls start early
                        nc.sync.dma_start(out=xqt[:, 0:4, :], in_=xq_r[:, 0:4, ms])
                        nc.sync.dma_start(
                            out=wq_t[:, 4:KT, :], in_=xq_like_w(wqg)[:, 4:KT, :]
                        )
                        nc.sync.dma_start(out=xqt[:, 4:KT, :], in_=xq_r[:, 4:KT, ms])
                        nc.sync.dma_start(
                            out=wk_t[:, 0:4, :], in_=xq_like_w(wkg)[:, 0:4, :]
                        )
                    else:
                        nc.sync.dma_start(out=xqt, in_=xq_r[:, :, ms])
                    if m == 0:
                        nc.sync.dma_start(out=xkt[:, 0:4, :], in_=xk_r[:, 0:4, ms])
                        nc.sync.dma_start(
                            out=wk_t[:, 4:KT, :], in_=xq_like_w(wkg)[:, 4:KT, :]
                        )
                        nc.sync.dma_start(out=xkt[:, 4:KT, :], in_=xk_r[:, 4:KT, ms])
                    else:
                        nc.sync.dma_start(out=xkt, in_=xk_r[:, :, ms])
                    if m == 0:
                        nc.sync.dma_start(out=wv_t, in_=xq_like_w(wvg))
                    nc.sync.dma_start(out=xvt, in_=xv_r[:, :, ms])
                    if m == 0:
                        nc.sync.dma_start(
                            out=wo_t, in_=wog[:].rearrange("(t p) n -> p t n", p=128)
                        )
                    xq_ts.append(xqt)
                    xk_ts.append(xkt)
                    xv_ts.append(xvt)

                for m in range(MS):
                    load_m(m)

                # ---- persistent activations ----
                qht = big.tile([128, 2, S], f16, tag="qht")
                kht = big.tile([128, 2, S], f16, tag="kht")
                vh = big.tile([128, JT, HG, DH + 1], bf16, tag="vh")
                ct = big.tile([128, 2, S], bf16, tag="ct")
                vh_ones_stage = wpool.tile([128, JT, HG, 1], f32, tag="vh_ones_st")
                nc.vector.memset(vh_ones_stage, 1.0)
                nc.scalar.activation(
                    out=vh[:, :, :, DH : DH + 1], in_=vh_ones_stage, func=Act.Copy
                )

                # ---- projection chunk emitters (consumed as fillers) ----
                def qk_chunks(m):
                    """QK projection chunk callables for m-slice m."""
                    ms = slice(m * 512, (m + 1) * 512)
                    qk_chunks.psums = {}

                    def qk_chunk(xts, w_t, b_t, dst, n, kks):
                        def emit():
                            if kks[0] == 0:
                                psum = po.tile([128, 512], f32, tag="po", name="qkp")
                                qk_chunks.psums[(id(xts), n)] = psum
                            psum = qk_chunks.psums[(id(xts), n)]
                            for kk in kks:
                                nc.tensor.matmul(
                                    psum,
                                    w_t[:, kk, n * 128 : (n + 1) * 128],
                                    xts[:, kk, :],
                                    start=(kk == 0),
                                    stop=(kk == KT - 1),
                                )
                            if kks[-1] == KT - 1:
                                nc.vector.tensor_scalar_add(
                                    dst[:, n, ms], psum, b_t[:, n, :]
                                )

                        return emit

                    out = []
                    splits = (
                        [list(range(0, 4)), list(range(4, KT))]
                        if m == 0
                        else [list(range(KT))]
                    )
                    for n in range(2):
                        for kks in splits:
                            out.append(qk_chunk(xq_ts[m], wq_t, bq_t, qht, n, kks))
                    for n in range(2):
                        for kks in splits:
                            out.append(qk_chunk(xk_ts[m], wk_t, bk_t, kht, n, kks))
                    return out

                def v_chunks(m):
                    def v_chunk(jj):
                        def emit():
                            j = m * 4 + jj
                            psum = po.tile([128, GD], f32, tag="po", name="vps")
                            for kk in range(KT):
                                nc.tensor.matmul(
                                    psum,
                                    xv_ts[m][:, kk, jj * 128 : (jj + 1) * 128],
                                    wv_t[:, kk, :],
                                    start=(kk == 0),
                                    stop=(kk == KT - 1),
                                )
                            nc.vector.tensor_copy(
                                vh[:, j, :, 0:DH],
                                psum[:].rearrange("p (h d) -> p h d", h=HG),
                            )

                        return emit

                    return [v_chunk(jj) for jj in range(4)]

                def outproj_its(IS):
                    def mk(it):
                        def emit():
                            r0 = IS * 512 + it * 128
                            out_sb = osb.tile([128, D], bf16, tag="out")
                            for nn in range(2):
                                o_psum = po.tile([128, 512], f32, tag="po")
                                for t in range(2):
                                    nc.tensor.matmul(
                                        o_psum,
                                        ct[:, t, r0 : r0 + 128],
                                        wo_t[:, t, nn * 512 : (nn + 1) * 512],
                                        start=(t == 0),
                                        stop=(t == 1),
                                    )
                                nc.vector.tensor_copy(
                                    out_sb[:, nn * 512 : (nn + 1) * 512], o_psum
                                )
                            nc.gpsimd.dma_start(out=outp[r0 : r0 + 128, :], in_=out_sb)

                        return emit

                    return [mk(it) for it in range(4)]

                # ---- attention ----
                def emit_attention(IS, filler, post_evac=None):
                    """Attention for i-slice IS in two passes. Pass A: per J,
                    scores -> exp -> attnV for q-tiles 0-1 (u in 2 PSUM
                    banks, scores double-buffered so ACT streams exps
                    back-to-back). Pass B: replay the stored pt tiles for
                    q-tiles 2-3 — pure PE work, no ACT dependency. filler()
                    interleaves other PE work; post_evac(t) is called right
                    after q-tile t's ct rows are complete."""
                    i0 = IS * 512
                    n_j = (IS + 1) * 4
                    u_banks = {}

                    def evac(t):
                        r0 = i0 + t * 128
                        dn = small.tile([128, HG], f32, tag="dn", name="dn")
                        nc.vector.tensor_copy(dn, u_banks[t][:, :, DH])
                        rc = small.tile([128, HG], f32, tag="rc", name="rc")
                        nc.vector.reciprocal_approx_fast(out=rc, in_=dn)
                        ctq = ctqp.tile([128, GD], bf16, tag="ctq")
                        for h in range(HG):
                            nc.vector.tensor_scalar_mul(
                                ctq[:, h * DH : (h + 1) * DH],
                                u_banks[t][:, h, 0:DH],
                                rc[:, h : h + 1],
                            )
                        for half in range(2):
                            tp = po.tile([128, 128], bf16, tag="po", name="tp")
                            nc.tensor.transpose(
                                tp, ctq[:, half * 128 : (half + 1) * 128], ident
                            )
                            nc.vector.tensor_copy(ct[:, half, r0 : r0 + 128], tp)
                        if post_evac is not None:
                            post_evac(t)

                    # start=True resets the accumulation state of the WHOLE
                    # psum bank (HW-probed), so only the first matmul into
                    # each u bank generation may carry it; later slots write
                    # fresh regions via has_written=False.
                    u_started = set()

                    def attnv(pt, J, h, ts):
                        for t in ts:
                            nc.tensor.matmul(
                                u_banks[t][:, h, :],
                                pt[:, h % 2, t * 128 : (t + 1) * 128],
                                vh[:, J, h, :],
                                start=(t not in u_started),
                                stop=(J == n_j - 4 + t),
                                skip_group_check=True,
                            )
                            u_started.add(t)

                    # pass A: scores + exp + attnV for q-tiles 0,1
                    for t in (0, 1):
                        u_banks[t] = psU.tile(
                            [128, HG, DH + 1], f32, tag="u", name=f"u{t}"
                        )
                    pts = []
                    for J in range(n_j):
                        diag = J >= n_j - 4
                        jd = J - (n_j - 4)
                        r = J * 128 - i0 if diag else 0
                        pt_j = []
                        for hp in range(2):
                            s_psum = ps.tile([128, 2, 512], f32, tag="ps")
                            for e in range(2):
                                lo = 64 * e
                                nc.tensor.matmul(
                                    s_psum[:, e, r:512],
                                    kht[lo : lo + DH, hp, J * 128 : (J + 1) * 128],
                                    qht[lo : lo + DH, hp, i0 + r : i0 + 512],
                                    start=True,
                                    stop=True,
                                )
                            pt = ppool.tile([128, 2, 512], bf16, tag="pt")
                            nc.scalar.activation(
                                out=pt[:, :, r:512],
                                in_=s_psum[:, :, r:512],
                                func=Act.Exp,
                            )
                            if diag:
                                nc.gpsimd.affine_select(
                                    out=pt[:, :, r : r + 128],
                                    in_=pt[:, :, r : r + 128],
                                    compare_op=Alu.is_ge,
                                    fill=0.0,
                                    base=0,
                                    pattern=[[0, 2], [1, 128]],
                                    channel_multiplier=-1,
                                )
                            pt_j.append(pt)
                            for e in range(2):
                                h = 2 * hp + e
                                ts = [t for t in (0, 1) if not diag or t >= jd]
                                attnv(pt, J, h, ts)
                            filler()
                        pts.append(pt_j)
                        if diag and jd in (0, 1):
                            evac(jd)
                        filler()

                    # pass B: attnV for q-tiles 2,3 from stored pt tiles
                    for t in (2, 3):
                        u_banks[t] = psU.tile(
                            [128, HG, DH + 1], f32, tag="u", name=f"u{t}"
                        )
                    for J in range(n_j):
                        diag = J >= n_j - 4
                        jd = J - (n_j - 4)
                        for hp in range(2):
                            for e in range(2):
                                h = 2 * hp + e
                                ts = [t for t in (2, 3) if not diag or t >= jd]
                                attnv(pts[J][hp], J, h, ts)
                        filler()
                    evac(2)
                    evac(3)

                # ---- main schedule ----
                #   B0: att(0) + proj(m1)
                #   B1: att(1) + proj(m2)
                #   B2: att(2) + proj(m3) QK + outproj(0)
                #   B3: att(3) + proj(m3) V + outproj(1,2); outproj(3) tiles
                #       emitted right after each q-tile evac
                for emit in qk_chunks(0) + v_chunks(0):
                    emit()

                blocks = [
                    qk_chunks(1) + v_chunks(1),
                    qk_chunks(2) + v_chunks(2),
                    qk_chunks(3),
                    v_chunks(3) + outproj_its(0) + outproj_its(1) + outproj_its(2),
                ]
                op3 = outproj_its(3)
                for IS, fill in enumerate(blocks):
                    n_j = (IS + 1) * 4
                    slots = 3 * n_j + 2
                    state = {"done": 0, "calls": 0}

                    def filler(fill=fill, state=state, slots=slots):
                        # paced: spread the block's filler items evenly over
                        # this block's filler slots so late ACT-bound J-steps
                        # still have PE work queued
                        state["calls"] += 1
                        target = state["calls"] * len(fill) / slots
                        while state["done"] < len(fill) and state["done"] < target:
                            fill[state["done"]]()
                            state["done"] += 1

                    post = (lambda t: op3[t]()) if IS == IST - 1 else None
                    emit_attention(IS, filler, post_evac=post)
                    while state["done"] < len(fill):
                        fill[state["done"]]()
                        state["done"] += 1
                if dbg:
                    nc.sync.dma_start(out=ctd[:, :, :], in_=ct)

            for _rep in range(rep):
                emit_body()

    nc.compile()
    return nc


def xq_like_w(w):
    return w[:].rearrange("(kt p) n -> p kt n", p=128)


def _get_nc():
    global _cached
    if _cached is None:
        _cached = _build()
    return _cached


def _in_maps(q, k, v, wq, bq, wk, bk, wv, bv, wo, bo):
    import ml_dtypes

    bf = ml_dtypes.bfloat16
    ident = np.eye(128, dtype=bf)
    maps = []
    for c in range(8):
        b, g = c // G, c % G
        cs = slice(g * GD, (g + 1) * GD)
        maps.append(
            {
                "xq": np.ascontiguousarray(q[b].T).astype(np.float16),
                "xk": np.ascontiguousarray(k[b].T).astype(np.float16),
                "xv": np.ascontiguousarray(v[b].T).astype(bf),
                "wqg": np.ascontiguousarray(wq[:, cs]).astype(np.float16),
                "wkg": np.ascontiguousarray(wk[:, cs]).astype(np.float16),
                "wvg": np.ascontiguousarray(wv[:, cs]).astype(bf),
                "wog": np.ascontiguousarray(wo[cs, :]).astype(bf),
                "bqg": np.ascontiguousarray(bq[cs]).reshape(2, 128, 1),
                "bkg": np.ascontiguousarray(bk[cs]).reshape(2, 128, 1),
                "identg": ident,
            }
        )
    return maps


def run(inputs, trace=False, trace_kwargs=None):
    from concourse.bass_utils import run_bass_kernel_spmd

    nc = _get_nc()
    maps = _in_maps(**inputs)
    res = run_bass_kernel_spmd(
        nc, maps, list(range(8)), trace=trace, **(trace_kwargs or {})
    )
    out = np.zeros((B, S, D), np.float32)
    for c in range(8):
        out[c // G] += res.results[c]["outp"].astype(np.float32)
    # exact bias fold: C = U/colsum + 1 (x) bv  =>  out += bv @ wo + bo
    out += inputs["bv"].astype(np.float32) @ inputs["wo"].astype(np.float32)
    out += inputs["bo"].astype(np.float32)
    return out.astype(np.float32), res


def kernel(**inputs) -> np.ndarray:
    out, _ = run(inputs)
    return out
